# revision 1
# baseline (speedup 1.0000x reference)
"""Trainium2 Bass kernel for nn_ChannelGroupConvUneven.

Computes, for full inputs
    x      (8, 256, 128, 128) f32
    weight (320, 256, 3, 3)   f32
    bias   (320,)             f32
    param  (5,)               i32   per-group input-channel thresholds
the reference
    out = conv2d(x, weight * mask(param), stride 1, VALID) + bias
    out shape (8, 320, 126, 126) f32
where mask zeroes weight[o, i] for i < param[o // 64].

Strategy: data-parallel over batch — one image per NeuronCore (8 cores),
weights/bias replicated. Weight masking + transposition to the matmul lhsT
layout happens on the host (it is tiny, and makes the group masking exact for
any runtime `param`). Each core then runs a dense 3x3 conv: per output tile
(an output-channel block x 4 output rows), 18 matmuls (2 cin blocks x 9 taps)
accumulate in one fp32 PSUM bank, evacuated by the scalar engine with a fused
per-channel bias add and DMA'd out. Matmuls run in float32r (the PE's
TF32-like fast-fp32 mode: ~4x the plain-fp32 rate at free-dim >= 256;
measured rel err vs the fp32 reference ~1.3e-4). Input rows stream in bands
that are double-buffered so the PE never waits on DMA after startup.

Notes from hardware measurements (see also the fallback switch below):
  * float32r matmuls must write PSUM starting at partition 0 (ISA check
    `s3d3_mm_valid_dst_partition`), so the 64-wide last channel block cannot
    be column-paired with tile_position; it simply runs at half array width
    (the ~17% loss this causes is the price of fp32-class accuracy).
  * Measured steady-state pacing: ~232 ns per N=504 matmul (2268 matmuls per
    image -> ~400 us/core), PE gapless outside a ~14 us startup and ~11 us
    kernel drain tail.
"""

import numpy as np

import concourse.mybir as mybir
import concourse.tile as tile
from concourse import bacc
from concourse.bass_utils import run_bass_kernel_spmd


def _ensure_axon_ntff_hook():
    """Best-effort: register the axon NTFF profile hook if the image's
    `antenv` stub lacks `axon_hooks` (concourse's trace path imports it
    unconditionally when BASS_TRACE is set). Purely optional — failures are
    ignored and tracing is simply unavailable."""
    try:
        import sys
        import types

        import antenv

        if "antenv.axon_hooks" in sys.modules:
            return
        mod = types.ModuleType("antenv.axon_hooks")
        _hook = [None]
        mod.set_axon_ntff_profile_hook = lambda h: _hook.__setitem__(0, h)
        mod.get_axon_ntff_profile_hook = lambda: _hook[0]
        sys.modules["antenv.axon_hooks"] = mod
        antenv.axon_hooks = mod
        from trn_agent_boot.trn_boot import _ntff_profile_via_ctypes

        mod.set_axon_ntff_profile_hook(
            _ntff_profile_via_ctypes("/opt/axon/libaxon_pjrt.so")
        )
    except Exception:
        pass


_ensure_axon_ntff_hook()

N_CORES = 8
P = 128
CIN, COUT, KH, KW = 256, 320, 3, 3
H = W = 128
HO = WO = 126
CB = CIN // P  # 2 cin blocks
NTAP = CB * KH * KW  # 18 accumulated matmuls per output tile

# output row tiles: 30 of 4 rows + 2 of 3 rows (N = 504 / 378 matmul free
# size, both >= 256 so float32r runs at full rate and both fit one PSUM
# bank). Grouped into bands of <= 6 tiles whose input rows are DMA'd
# together (double-buffered).
TILES = [(r, 4) for r in range(0, 120, 4)] + [(120, 3), (123, 3)]
BANDS = [TILES[i : i + 6] for i in range(0, len(TILES), 6)]

CO_BLOCKS = [(0, 128, 0), (128, 128, 1), (256, 64, 2)]  # (co0, width, bias col)

# float32r: PE "fast fp32" mode (TF32-like rounding, fp32 PSUM accumulation),
# ~4x the plain-fp32 matmul rate. Measured rel err ~1.3e-4 vs the fp32
# reference. Set to mybir.dt.float32 for full fp32 accuracy (measured
# 3.7e-7 rel err) at ~4.1x the runtime (~1.75 ms/core vs ~0.42 ms/core).
MM_DT = mybir.dt.float32r

_NC_CACHE = {}


def _build_nc(mm_dt):
    nc = bacc.Bacc("TRN2", target_bir_lowering=False, debug=False)
    f32 = mybir.dt.float32

    x_d = nc.dram_tensor("x", [CIN, H, W], mm_dt, kind="ExternalInput").ap()
    w_d = nc.dram_tensor(
        "wt", [P, CB, KH, KW, COUT], mm_dt, kind="ExternalInput"
    ).ap()
    b_d = nc.dram_tensor("biasp", [P, 3], f32, kind="ExternalInput").ap()
    o_d = nc.dram_tensor("out", [COUT, HO, WO], f32, kind="ExternalOutput").ap()

    # x viewed as [p, cb, h, w]: cin = cb*128 + p
    x_re = x_d.rearrange("(cb p) h w -> p cb h w", p=P)

    with tile.TileContext(nc) as tc:
        with (
            tc.tile_pool(name="wpool", bufs=1) as wpool,
            tc.tile_pool(name="xpool", bufs=3) as xpool,
            tc.tile_pool(name="opool", bufs=6) as opool,
            tc.tile_pool(name="psum", bufs=8, space="PSUM") as psum_pool,
        ):
            wt = wpool.tile([P, CB, KH, KW, COUT], mm_dt)
            bt = wpool.tile([P, 3], f32)

            def rhs(xb, in_r0, r, rpt, cb, dy, dx):
                rr = r - in_r0 + dy
                return xb[:, cb, rr : rr + rpt, dx : dx + WO]

            for band_idx, band in enumerate(BANDS):
                in_r0 = band[0][0]
                in_rows = band[-1][0] + band[-1][1] + 2 - in_r0
                xb = xpool.tile([P, CB, in_rows, W], mm_dt, tag="xband")
                # Band 0's input rows, the weights, and the bias are split
                # across both HWDGE queues (sync + scalar) and chunked so the
                # first tiles' matmuls start as soon as their slices land
                # (subtile deps). Queue order matters: each queue drains in
                # program order, so the first tile's needs go first. Later
                # bands prefetch on the scalar queue while output stores run
                # on sync.
                if band_idx == 0:
                    # staged so the first matmul only needs rows 0:6 + the
                    # first weight chunk; the HAM cold-clock ramp (~420 ns/MM
                    # for the first ~12 matmuls) buys time for the rest.
                    for cb in range(CB):
                        eng = nc.sync if cb == 0 else nc.scalar
                        eng.dma_start(
                            xb[:, cb, 0:6], x_re[:, cb, in_r0 : in_r0 + 6, :]
                        )
                    nc.scalar.dma_start(bt[:], b_d[:])
                    nc.sync.dma_start(wt[:, 0, 0], w_d[:, 0, 0])
                    nc.scalar.dma_start(wt[:, 1, 0], w_d[:, 1, 0])
                    for cb in range(CB):
                        eng = nc.sync if cb == 0 else nc.scalar
                        eng.dma_start(
                            xb[:, cb, 6:14], x_re[:, cb, in_r0 + 6 : in_r0 + 14, :]
                        )
                        eng.dma_start(
                            xb[:, cb, 14:in_rows],
                            x_re[:, cb, in_r0 + 14 : in_r0 + in_rows, :],
                        )
                    for dy in range(1, KH):
                        nc.sync.dma_start(wt[:, 0, dy], w_d[:, 0, dy])
                        nc.scalar.dma_start(wt[:, 1, dy], w_d[:, 1, dy])
                else:
                    nc.scalar.dma_start(
                        xb[:], x_re[:, :, in_r0 : in_r0 + in_rows, :]
                    )

                for cob_i, (co0, com, bcol) in enumerate(CO_BLOCKS):
                    if band_idx == 0 and cob_i == 0:
                        # Warm-up sweep: the weight chunks are still streaming
                        # in, and tile-major order would burn each (cb, dy)
                        # chunk in ~0.7us while chunks arrive ~2us apart.
                        # Going chunk-major across all 6 row tiles gives each
                        # chunk ~4us of work, so the PE never stalls on the
                        # weight DMA.
                        pss = [
                            psum_pool.tile(
                                [P, rpt, WO], f32, tag="ps", name=f"ps_warm{ti}"
                            )
                            for ti, (r, rpt) in enumerate(band)
                        ]
                        for cb in range(CB):
                            for dy in range(KH):
                                for ti, (r, rpt) in enumerate(band):
                                    for dx in range(KW):
                                        nc.tensor.matmul(
                                            pss[ti][:com],
                                            wt[:, cb, dy, dx, co0 : co0 + com],
                                            rhs(xb, in_r0, r, rpt, cb, dy, dx),
                                            start=(cb == 0 and dy == 0 and dx == 0),
                                            stop=(
                                                cb == CB - 1
                                                and dy == KH - 1
                                                and dx == KW - 1
                                            ),
                                        )
                        for ti, (r, rpt) in enumerate(band):
                            ot = opool.tile([P, rpt, WO], f32, tag="ot")
                            nc.scalar.add(
                                ot[:com], pss[ti][:com], bt[:com, bcol : bcol + 1]
                            )
                            nc.sync.dma_start(
                                o_d[co0 : co0 + com, r : r + rpt, :], ot[:com]
                            )
                        continue
                    for r, rpt in band:
                        ps = psum_pool.tile([P, rpt, WO], f32, tag="ps")
                        k = 0
                        for cb in range(CB):
                            for dy in range(KH):
                                for dx in range(KW):
                                    nc.tensor.matmul(
                                        ps[:com],
                                        wt[:, cb, dy, dx, co0 : co0 + com],
                                        rhs(xb, in_r0, r, rpt, cb, dy, dx),
                                        start=(k == 0),
                                        stop=(k == NTAP - 1),
                                    )
                                    k += 1
                        ot = opool.tile([P, rpt, WO], f32, tag="ot")
                        # evacuate PSUM -> SBUF with fused per-channel bias add
                        nc.scalar.add(
                            ot[:com], ps[:com], bt[:com, bcol : bcol + 1]
                        )
                        nc.sync.dma_start(
                            o_d[co0 : co0 + com, r : r + rpt, :], ot[:com]
                        )
    nc.compile()
    return nc


def _get_nc():
    key = str(MM_DT)
    if key not in _NC_CACHE:
        _NC_CACHE[key] = _build_nc(MM_DT)
    return _NC_CACHE[key]


def _preprocess(x, weight, bias, param):
    x = np.ascontiguousarray(np.asarray(x), dtype=np.float32)
    weight = np.asarray(weight, dtype=np.float32)
    bias = np.asarray(bias, dtype=np.float32)
    param = np.asarray(param)

    # host-side weight masking (group g of 64 output channels uses cin >= param[g])
    thresh = np.repeat(param.astype(np.int64), COUT // param.shape[0])  # [COUT]
    mask = (np.arange(CIN)[None, :] >= thresh[:, None]).astype(np.float32)
    wm = weight * mask[:, :, None, None]
    # lhsT layout: [p, cb, kh, kw, cout]
    wT = np.ascontiguousarray(
        wm.reshape(COUT, CB, P, KH, KW).transpose(2, 1, 3, 4, 0)
    )
    biasp = np.zeros((P, 3), np.float32)
    biasp[:, 0] = bias[0:128]
    biasp[:, 1] = bias[128:256]
    biasp[:64, 2] = bias[256:320]
    return x, wT, biasp


def kernel(x, weight, bias, param):
    x, wT, biasp = _preprocess(x, weight, bias, param)
    nc = _get_nc()
    in_maps = [{"x": x[i], "wt": wT, "biasp": biasp} for i in range(N_CORES)]
    res = run_bass_kernel_spmd(nc, in_maps, core_ids=list(range(N_CORES)))
    return np.stack([r["out"] for r in res.results], axis=0)



# revision 4
# speedup vs baseline: 1.1896x; 1.1896x over previous
"""Trainium2 Bass kernel for nn_ChannelGroupConvUneven.

Computes, for full inputs
    x      (8, 256, 128, 128) f32
    weight (320, 256, 3, 3)   f32
    bias   (320,)             f32
    param  (5,)               i32   per-group input-channel thresholds
the reference
    out = conv2d(x, weight * mask(param), stride 1, VALID) + bias
    out shape (8, 320, 126, 126) f32
where mask zeroes weight[o, i] for i < param[o // 64].

Strategy: data-parallel over batch — one image per NeuronCore (8 cores),
weights/bias replicated. Weight masking + transposition to the matmul lhsT
layout happens on the host (it is tiny, and makes the group masking exact for
any runtime `param`). Each core then runs a dense 3x3 conv: per output tile
(an output-channel block x 4 output rows), 18 matmuls (2 cin blocks x 9 taps)
accumulate in one fp32 PSUM bank, evacuated by the scalar engine with a fused
per-channel bias add and DMA'd out. Matmuls run in float32r (the PE's
TF32-like fast-fp32 mode: ~4x the plain-fp32 rate at free-dim >= 256;
measured rel err vs the fp32 reference ~1.3e-4). Input rows stream in bands
that are double-buffered so the PE never waits on DMA after startup.

Notes from hardware measurements (see also the fallback switch below):
  * float32r matmuls must write PSUM starting at partition 0 (ISA check
    `s3d3_mm_valid_dst_partition`), so the 64-wide last channel block cannot
    be column-paired with tile_position; it simply runs at half array width
    (the ~17% loss this causes is the price of fp32-class accuracy).
  * Measured steady-state pacing: ~232 ns per N=504 matmul (2268 matmuls per
    image -> ~400 us/core), PE gapless outside a ~14 us startup and ~11 us
    kernel drain tail.
"""

import ml_dtypes
import numpy as np

import concourse.mybir as mybir
import concourse.tile as tile
from concourse import bacc
from concourse.bass_utils import run_bass_kernel_spmd


def _ensure_axon_ntff_hook():
    """Best-effort: register the axon NTFF profile hook if the image's
    `antenv` stub lacks `axon_hooks` (concourse's trace path imports it
    unconditionally when BASS_TRACE is set). Purely optional — failures are
    ignored and tracing is simply unavailable."""
    try:
        import sys
        import types

        import antenv

        if "antenv.axon_hooks" in sys.modules:
            return
        mod = types.ModuleType("antenv.axon_hooks")
        _hook = [None]
        mod.set_axon_ntff_profile_hook = lambda h: _hook.__setitem__(0, h)
        mod.get_axon_ntff_profile_hook = lambda: _hook[0]
        sys.modules["antenv.axon_hooks"] = mod
        antenv.axon_hooks = mod
        from trn_agent_boot.trn_boot import _ntff_profile_via_ctypes

        mod.set_axon_ntff_profile_hook(
            _ntff_profile_via_ctypes("/opt/axon/libaxon_pjrt.so")
        )
    except Exception:
        pass


_ensure_axon_ntff_hook()

N_CORES = 8
P = 128
CIN, COUT, KH, KW = 256, 320, 3, 3
H = W = 128
HO = WO = 126
CB = CIN // P  # 2 cin blocks
NTAP = CB * KH * KW  # 18 accumulated matmuls per output tile

# output row tiles: 30 of 4 rows + 2 of 3 rows (N = 504 / 378 matmul free
# size, both >= 256 so float32r runs at full rate and both fit one PSUM
# bank). Grouped into bands of <= 6 tiles whose input rows are DMA'd
# together (double-buffered).
TILES = [(r, 4) for r in range(0, 120, 4)] + [(120, 3), (123, 3)]
BANDS = [TILES[i : i + 6] for i in range(0, len(TILES), 6)]

CO_BLOCKS = [(0, 128, 0), (128, 128, 1), (256, 64, 2)]  # (co0, width, bias col)

# Matmul operand dtype. All run at 1 PE column/cycle for N >= 256, but only
# non-fp32 weights get Fast Weight Load (FWL): with float32r the per-matmul
# LDWEIGHTS (~187 ns, measured) is partially exposed, pacing matmuls at
# ~254 ns instead of ~210 ns. bfloat16 keeps fp32 PSUM accumulation; with
# 2304-term dot products the measured rel err stays ~1e-3 (tolerance 2e-2).
# Fallbacks: mybir.dt.float32r (rel err ~1.3e-4), mybir.dt.float32 (exact,
# ~4x slower).
MM_DT = mybir.dt.bfloat16
_NP_MM_DT = {
    mybir.dt.bfloat16: ml_dtypes.bfloat16,
    mybir.dt.float32r: np.float32,
    mybir.dt.float32: np.float32,
}

_NC_CACHE = {}


def _build_nc(mm_dt):
    nc = bacc.Bacc("TRN2", target_bir_lowering=False, debug=False)
    f32 = mybir.dt.float32

    x_d = nc.dram_tensor("x", [CIN, H, W], mm_dt, kind="ExternalInput").ap()
    w_d = nc.dram_tensor(
        "wt", [P, CB, KH, KW, COUT], mm_dt, kind="ExternalInput"
    ).ap()
    b_d = nc.dram_tensor("biasp", [P, 3], f32, kind="ExternalInput").ap()
    o_d = nc.dram_tensor("out", [COUT, HO, WO], f32, kind="ExternalOutput").ap()

    # x viewed as [p, cb, h, w]: cin = cb*128 + p
    x_re = x_d.rearrange("(cb p) h w -> p cb h w", p=P)

    with tile.TileContext(nc) as tc:
        with (
            tc.tile_pool(name="wpool", bufs=1) as wpool,
            tc.tile_pool(name="xpool", bufs=3) as xpool,
            tc.tile_pool(name="opool", bufs=6) as opool,
            tc.tile_pool(name="psum", bufs=8, space="PSUM") as psum_pool,
        ):
            wt = wpool.tile([P, CB, KH, KW, COUT], mm_dt)
            bt = wpool.tile([P, 3], f32)

            def rhs(xb, in_r0, r, rpt, cb, dy, dx):
                rr = r - in_r0 + dy
                return xb[:, cb, rr : rr + rpt, dx : dx + WO]

            for band_idx, band in enumerate(BANDS):
                in_r0 = band[0][0]
                in_rows = band[-1][0] + band[-1][1] + 2 - in_r0
                xb = xpool.tile([P, CB, in_rows, W], mm_dt, tag="xband")
                # Band 0's input rows, the weights, and the bias are split
                # across both HWDGE queues (sync + scalar) and chunked so the
                # first tiles' matmuls start as soon as their slices land
                # (subtile deps). Queue order matters: each queue drains in
                # program order, so the first tile's needs go first. Later
                # bands prefetch on the scalar queue while output stores run
                # on sync.
                if band_idx == 0:
                    # staged so the first matmul only needs rows 0:6 + the
                    # first weight chunk; the HAM cold-clock ramp (~420 ns/MM
                    # for the first ~12 matmuls) buys time for the rest.
                    for cb in range(CB):
                        eng = nc.sync if cb == 0 else nc.scalar
                        eng.dma_start(
                            xb[:, cb, 0:6], x_re[:, cb, in_r0 : in_r0 + 6, :]
                        )
                    nc.scalar.dma_start(bt[:], b_d[:])
                    nc.sync.dma_start(wt[:, 0, 0], w_d[:, 0, 0])
                    nc.scalar.dma_start(wt[:, 1, 0], w_d[:, 1, 0])
                    for cb in range(CB):
                        eng = nc.sync if cb == 0 else nc.scalar
                        eng.dma_start(
                            xb[:, cb, 6:14], x_re[:, cb, in_r0 + 6 : in_r0 + 14, :]
                        )
                        eng.dma_start(
                            xb[:, cb, 14:in_rows],
                            x_re[:, cb, in_r0 + 14 : in_r0 + in_rows, :],
                        )
                    for dy in range(1, KH):
                        nc.sync.dma_start(wt[:, 0, dy], w_d[:, 0, dy])
                        nc.scalar.dma_start(wt[:, 1, dy], w_d[:, 1, dy])
                else:
                    nc.scalar.dma_start(
                        xb[:], x_re[:, :, in_r0 : in_r0 + in_rows, :]
                    )

                for cob_i, (co0, com, bcol) in enumerate(CO_BLOCKS):
                    if band_idx == 0 and cob_i == 0:
                        # Warm-up sweep: the weight chunks are still streaming
                        # in, and tile-major order would burn each (cb, dy)
                        # chunk in ~0.7us while chunks arrive ~2us apart.
                        # Going chunk-major across all 6 row tiles gives each
                        # chunk ~4us of work, so the PE never stalls on the
                        # weight DMA.
                        pss = [
                            psum_pool.tile(
                                [P, rpt, WO], f32, tag="ps", name=f"ps_warm{ti}"
                            )
                            for ti, (r, rpt) in enumerate(band)
                        ]
                        for cb in range(CB):
                            for dy in range(KH):
                                for ti, (r, rpt) in enumerate(band):
                                    for dx in range(KW):
                                        nc.tensor.matmul(
                                            pss[ti][:com],
                                            wt[:, cb, dy, dx, co0 : co0 + com],
                                            rhs(xb, in_r0, r, rpt, cb, dy, dx),
                                            start=(cb == 0 and dy == 0 and dx == 0),
                                            stop=(
                                                cb == CB - 1
                                                and dy == KH - 1
                                                and dx == KW - 1
                                            ),
                                        )
                        for ti, (r, rpt) in enumerate(band):
                            ot = opool.tile([P, rpt, WO], f32, tag="ot")
                            nc.scalar.add(
                                ot[:com], pss[ti][:com], bt[:com, bcol : bcol + 1]
                            )
                            nc.sync.dma_start(
                                o_d[co0 : co0 + com, r : r + rpt, :], ot[:com]
                            )
                        continue
                    for r, rpt in band:
                        ps = psum_pool.tile([P, rpt, WO], f32, tag="ps")
                        k = 0
                        for cb in range(CB):
                            for dy in range(KH):
                                for dx in range(KW):
                                    nc.tensor.matmul(
                                        ps[:com],
                                        wt[:, cb, dy, dx, co0 : co0 + com],
                                        rhs(xb, in_r0, r, rpt, cb, dy, dx),
                                        start=(k == 0),
                                        stop=(k == NTAP - 1),
                                    )
                                    k += 1
                        ot = opool.tile([P, rpt, WO], f32, tag="ot")
                        # evacuate PSUM -> SBUF with fused per-channel bias add
                        nc.scalar.add(
                            ot[:com], ps[:com], bt[:com, bcol : bcol + 1]
                        )
                        nc.sync.dma_start(
                            o_d[co0 : co0 + com, r : r + rpt, :], ot[:com]
                        )
    nc.compile()
    return nc


def _get_nc():
    key = str(MM_DT)
    if key not in _NC_CACHE:
        _NC_CACHE[key] = _build_nc(MM_DT)
    return _NC_CACHE[key]


def _preprocess(x, weight, bias, param):
    np_mm = _NP_MM_DT[MM_DT]
    x = np.ascontiguousarray(np.asarray(x, dtype=np.float32).astype(np_mm))
    weight = np.asarray(weight, dtype=np.float32)
    bias = np.asarray(bias, dtype=np.float32)
    param = np.asarray(param)

    # host-side weight masking (group g of 64 output channels uses cin >= param[g])
    thresh = np.repeat(param.astype(np.int64), COUT // param.shape[0])  # [COUT]
    mask = (np.arange(CIN)[None, :] >= thresh[:, None]).astype(np.float32)
    wm = weight * mask[:, :, None, None]
    # lhsT layout: [p, cb, kh, kw, cout]
    wT = np.ascontiguousarray(
        wm.reshape(COUT, CB, P, KH, KW).transpose(2, 1, 3, 4, 0).astype(np_mm)
    )
    biasp = np.zeros((P, 3), np.float32)
    biasp[:, 0] = bias[0:128]
    biasp[:, 1] = bias[128:256]
    biasp[:64, 2] = bias[256:320]
    return x, wT, biasp


def kernel(x, weight, bias, param):
    x, wT, biasp = _preprocess(x, weight, bias, param)
    nc = _get_nc()
    in_maps = [{"x": x[i], "wt": wT, "biasp": biasp} for i in range(N_CORES)]
    res = run_bass_kernel_spmd(nc, in_maps, core_ids=list(range(N_CORES)))
    return np.stack([r["out"] for r in res.results], axis=0)

